# revision 1
# baseline (speedup 1.0000x reference)
"""DeepseekV3.2 sparse MLA attention — Trainium2 Bass kernel, 8-core SPMD.

Sharding: queries row-sharded (256 rows/core); keys/values replicated.
Indexer k_idx/w_idx computed fp32 token-sharded + AllGather; indexer scores
via f16 hi/lo split matmuls (exact top-k mask); attention path f16.
Top-k realized as a per-row threshold found by bisection counting on the DVE.
"""
import numpy as np

import concourse.bass as bass
import concourse.bacc as bacc
import concourse.mybir as mybir
import concourse.tile as tile
import concourse.dve_ops as dve_ops_mod
from concourse.bass_utils import run_bass_kernel_spmd
from concourse.dve_spec import Spec, Src0, Src1, C0, relu, select, lower, spec_leaves
from concourse.dve_ops import DveOp, OPS, get_dve_sub_opcode
from concourse.dve_uop import DveOpSpec

dt = mybir.dt
AF = mybir.ActivationFunctionType
OP = mybir.AluOpType
AX = mybir.AxisListType

T, HID, H, NOPE, ROPE, VD = 2048, 7168, 16, 128, 64, 128
QLR, KVLR, IH, ID, TOPK = 1536, 512, 32, 128, 512
EPS = 1e-6
SCALE = float((NOPE + ROPE) ** -0.5)
ROPE_BASE = 10000.0
NC = 8
R = T // NC            # 256 local rows
NEG = -3.0e38
SEARCH_ITERS = 20
KT_H, KT_Q, KT_KV = HID // 128, QLR // 128, KVLR // 128


def _register(op):
    for o in OPS:
        if o.name == op.name:
            return o
    OPS.append(op)
    dve_ops_mod.CUSTOM_DVE_SPECS[op.name] = op.spec
    dve_ops_mod._SUB_OPCODE_FOR_NAME[op.name] = (
        dve_ops_mod._CUSTOM_DVE_ROW_BASE + len(OPS) - 1)
    for ver in ("v3", "v4"):
        sp = DveOpSpec(name=op.name, opcode=get_dve_sub_opcode(op.name),
                       uops=lower(op.spec, ver=ver),
                       rd1_en=Src1 in spec_leaves(op.spec))
        op.uops_sha[ver] = sp.sha(ver)
    return op


ACC_W_RELU = _register(DveOp(
    "ACC_W_RELU",
    Spec(body=Src1 + C0 * relu(Src0),
         reference=lambda in0, in1, s0, s1, imm2: in1 + s0 * np.maximum(in0, 0)),
    subdim=False, uops_sha={}))

MASKSEL = _register(DveOp(
    "MASKSEL",
    Spec(body=select(Src1, Src0, C0),
         reference=lambda in0, in1, s0, s1, imm2: np.where(in1 != 0, in0, s0)),
    subdim=False, uops_sha={}))


def build_nc():
    nc = bacc.Bacc(None)
    f32, f16 = dt.float32, dt.float16

    def din(name, shape, d=f32):
        return nc.dram_tensor(name, list(shape), d, kind="ExternalInput")

    qcT = din("qcT", [QLR, R])
    kvcT = din("kvcT", [KVLR, T])
    hT = din("hT", [HID, R])
    wikww = din("wikww", [HID, ID + IH])
    wiq_hi = din("wiq_hi", [QLR, IH * ID], f16)
    wiq_lo = din("wiq_lo", [QLR, IH * ID], f16)
    wq = din("wq", [QLR, H * (NOPE + ROPE)], f16)
    wkv_k = din("wkv_k", [KVLR, H * NOPE], f16)
    wkv_v = din("wkv_v", [KVLR, H * VD], f16)
    wo = din("wo", [H * VD, HID], f16)
    kperT = din("kperT", [ROPE, T], f16)
    cosT = din("cosT", [128, R])
    sinT = din("sinT", [128, R])
    cosK = din("cosK", [R, 32])
    sinK = din("sinK", [R, 32])
    gam_d = din("gam_rep", [128, ID])
    bet_d = din("bet_rep", [128, ID])
    idf32 = din("idf32", [128, 128])
    idf16 = din("idf16", [128, 128], f16)
    caus_d = din("causal01", [2, 128, T], f16)
    out_d = nc.dram_tensor("out", [R, HID], f32, kind="ExternalOutput")

    with tile.TileContext(nc) as tc:
        cst = tc.alloc_tile_pool(name="cst", bufs=1)
        ones16 = cst.tile([128, 128], f16); nc.vector.memset(ones16[:], 1.0)
        ones32 = cst.tile([128, 1], f32); nc.vector.memset(ones32[:], 1.0)
        onesrow = cst.tile([1, 128], f32); nc.vector.memset(onesrow[:], 1.0)
        id32 = cst.tile([128, 128], f32); nc.sync.dma_start(id32[:], idf32[:])
        id16 = cst.tile([128, 128], f16); nc.sync.dma_start(id16[:], idf16[:])
        cosT_s = cst.tile([128, R], f32); nc.sync.dma_start(cosT_s[:], cosT[:])
        sinT_s = cst.tile([128, R], f32); nc.sync.dma_start(sinT_s[:], sinT[:])
        kpe_s = cst.tile([ROPE, T], f16); nc.sync.dma_start(kpe_s[:], kperT[:])
        w_sb = cst.tile([128, 2, IH], f32)
        eps128 = cst.tile([128, 1], f32); nc.vector.memset(eps128[:], EPS)
        eps1 = cst.tile([1, 1], f32); nc.vector.memset(eps1[:], EPS)
        maskT = cst.tile([128, 16, R], f16)
        qhi = cst.tile([128, KT_Q, R], f16)
        kvn = cst.tile([128, KT_KV, T], f16)

        # ================= phase 1: indexer =================
        with tc.tile_pool(name="p1", bufs=1) as p1, \
             tc.tile_pool(name="str1", bufs=2) as st, \
             tc.tile_pool(name="tmp1", bufs=2) as tp, \
             tc.tile_pool(name="dram1", bufs=1, space="DRAM") as drp:
            scores = [p1.tile([128, T], f32, tag=f"sc{i}", name=f"sc{i}") for i in range(2)]
            mask_r = [p1.tile([128, T], f16, tag=f"mk{i}", name=f"mk{i}") for i in range(2)]
            kidxT = p1.tile([128, T], f32)
            khi = p1.tile([128, T], f16)
            klo = p1.tile([128, T], f16)
            qlo = p1.tile([128, KT_Q, R], f16)
            caus_s = [p1.tile([128, T], f16, tag=f"cz{i}", name=f"cz{i}") for i in range(2)]
            for rt in range(2):
                nc.sync.dma_start(caus_s[rt][:], caus_d[rt])
            cosk_s = p1.tile([128, 2, 32], f32)
            nc.sync.dma_start(cosk_s[:], cosK[:].rearrange("(m p) c -> p m c", p=128))
            sink_s = p1.tile([128, 2, 32], f32)
            nc.sync.dma_start(sink_s[:], sinK[:].rearrange("(m p) c -> p m c", p=128))
            gam_s = p1.tile([128, ID], f32); nc.sync.dma_start(gam_s[:], gam_d[:])
            bet_s = p1.tile([128, ID], f32); nc.sync.dma_start(bet_s[:], bet_d[:])
            qcT_s = p1.tile([128, KT_Q, R], f32)
            nc.sync.dma_start(qcT_s[:], qcT[:].rearrange("(k p) r -> p k r", p=128))
            kvcT_s = p1.tile([128, KT_KV, T], f32)
            nc.sync.dma_start(kvcT_s[:], kvcT[:].rearrange("(k p) t -> p k t", p=128))
            rrep = p1.tile([128, R], f32)
            rrep2 = p1.tile([128, T], f32)

            # ---- A: k_idx local fp32 + w_idx; AllGather; transpose ----
            inb = drp.tile([R, ID], f32)
            outb = drp.tile([T, ID], f32)
            with tc.tile_pool(name="ps_kw", bufs=2, space="PSUM") as ps_kw:
                for mt in range(2):
                    ps = ps_kw.tile([128, ID + IH], f32, tag="kw")
                    for kt in range(KT_H):
                        htile = st.tile([128, R], f32, tag="ht")
                        nc.sync.dma_start(htile[:], hT[kt * 128:(kt + 1) * 128, :])
                        wt = st.tile([128, ID + IH], f32, tag="wik")
                        nc.sync.dma_start(wt[:], wikww[kt * 128:(kt + 1) * 128, :])
                        nc.tensor.matmul(ps[:], htile[:, mt * 128:(mt + 1) * 128],
                                         wt[:], start=(kt == 0), stop=(kt == KT_H - 1))
                    nc.vector.tensor_copy(w_sb[:, mt, :], ps[:, ID:])
                    kl = tp.tile([128, ID], f32, tag="kl")
                    mu = tp.tile([128, 1], f32, tag="mu")
                    nc.vector.tensor_reduce(mu[:], ps[:, :ID], AX.X, OP.add)
                    nc.vector.tensor_scalar_mul(mu[:], mu[:], 1.0 / ID)
                    nc.vector.tensor_scalar(kl[:], ps[:, :ID], mu[:], None, OP.subtract)
                    ssq = tp.tile([128, 1], f32, tag="ssq")
                    sq = tp.tile([128, ID], f32, tag="sq")
                    nc.scalar.activation(sq[:], kl[:], AF.Square, accum_out=ssq[:])
                    std = tp.tile([128, 1], f32, tag="std")
                    nc.scalar.activation(std[:], ssq[:], AF.Sqrt, scale=1.0 / ID, bias=eps128[:])
                    rstd = tp.tile([128, 1], f32, tag="rstd")
                    nc.vector.reciprocal(rstd[:], std[:])
                    nc.vector.tensor_scalar(kl[:], kl[:], rstd[:], None, OP.mult)
                    nc.vector.tensor_mul(kl[:], kl[:], gam_s[:])
                    nc.vector.tensor_add(kl[:], kl[:], bet_s[:])
                    t1 = tp.tile([128, 32], f32, tag="kr1")
                    t2 = tp.tile([128, 32], f32, tag="kr2")
                    x1, x2 = kl[:, 0:32], kl[:, 32:64]
                    cc = cosk_s[:, mt, :]
                    ss2 = sink_s[:, mt, :]
                    nc.vector.tensor_mul(t1[:], x1, cc)
                    nc.vector.tensor_mul(t2[:], x2, ss2)
                    nc.vector.tensor_mul(x2, x2, cc)
                    nc.vector.tensor_mul(x1, x1, ss2)
                    nc.vector.tensor_add(x2, x2, x1)
                    nc.vector.tensor_sub(x1, t1[:], t2[:])
                    nc.sync.dma_start(inb[mt * 128:(mt + 1) * 128, :], kl[:])
                nc.gpsimd.collective_compute(
                    "AllGather", OP.bypass, replica_groups=[list(range(NC))],
                    ins=[inb[:]], outs=[outb[:]])
                for jt in range(16):
                    kb = st.tile([128, ID], f32, tag="kb")
                    nc.sync.dma_start(kb[:], outb[jt * 128:(jt + 1) * 128, :])
                    pt = ps_kw.tile([128, 128], f32, tag="ktr")
                    nc.tensor.transpose(pt[:], kb[:], id32[:])
                    nc.scalar.copy(kidxT[:, jt * 128:(jt + 1) * 128], pt[:])
                nc.vector.tensor_copy(khi[:], kidxT[:])
                nc.vector.tensor_sub(klo[:], kidxT[:], khi[:])

            # ---- B: rmsnorm qc (hi/lo) and kv (f16) ----
            with tc.tile_pool(name="ps_sm", bufs=1, space="PSUM") as ps_sm:
                sqp = ps_sm.tile([1, R], f32, tag="sqp")
                for kt in range(KT_Q):
                    sq = tp.tile([128, R], f32, tag="qsq")
                    nc.scalar.activation(sq[:], qcT_s[:, kt, :], AF.Square)
                    nc.tensor.matmul(sqp[:], ones32[:], sq[:],
                                     start=(kt == 0), stop=(kt == KT_Q - 1))
                stdq = tp.tile([1, R], f32, tag="stdq")
                nc.scalar.activation(stdq[:], sqp[:], AF.Sqrt, scale=1.0 / QLR, bias=eps1[:])
                rstdq = tp.tile([1, R], f32, tag="rstdq")
                nc.vector.reciprocal(rstdq[:], stdq[:])
                rp = ps_sm.tile([128, R], f32, tag="rp")
                nc.tensor.matmul(rp[:], onesrow[:], rstdq[:], start=True, stop=True)
                nc.scalar.copy(rrep[:], rp[:])
                for kt in range(KT_Q):
                    qn = tp.tile([128, R], f32, tag="qn")
                    nc.vector.tensor_mul(qn[:], qcT_s[:, kt, :], rrep[:])
                    nc.vector.tensor_copy(qhi[:, kt, :], qn[:])
                    nc.vector.tensor_sub(qlo[:, kt, :], qn[:], qhi[:, kt, :])
                for ncn in range(4):
                    js = ncn * 512
                    svp = ps_sm.tile([1, 512], f32, tag="svp")
                    for kt in range(KT_KV):
                        sq = tp.tile([128, 512], f32, tag="kvsq")
                        nc.scalar.activation(sq[:], kvcT_s[:, kt, js:js + 512], AF.Square)
                        nc.tensor.matmul(svp[:], ones32[:], sq[:],
                                         start=(kt == 0), stop=(kt == KT_KV - 1))
                    stdv = tp.tile([1, 512], f32, tag="stdv")
                    nc.scalar.activation(stdv[:], svp[:], AF.Sqrt,
                                         scale=1.0 / KVLR, bias=eps1[:])
                    rstdv = tp.tile([1, 512], f32, tag="rstdv")
                    nc.vector.reciprocal(rstdv[:], stdv[:])
                    rp2 = ps_sm.tile([128, 512], f32, tag="rp2")
                    nc.tensor.matmul(rp2[:], onesrow[:], rstdv[:], start=True, stop=True)
                    nc.scalar.copy(rrep2[:, js:js + 512], rp2[:])
                for kt in range(KT_KV):
                    nc.vector.tensor_mul(kvn[:, kt, :], kvcT_s[:, kt, :], rrep2[:])

            # ---- C: q_idx (3-term) + rope + split + logits + scores ----
            for i in range(2):
                nc.vector.memset(scores[i][:], 0.0)
            with tc.tile_pool(name="ps_qi", bufs=2, space="PSUM") as ps_qi, \
                 tc.tile_pool(name="ps_lg", bufs=2, space="PSUM") as ps_lg:
                for h in range(IH):
                    ps = ps_qi.tile([128, R], f32, tag="qi")
                    c0 = h * 128
                    wht = st.tile([128, KT_Q, 128], f16, tag="wiqh")
                    nc.sync.dma_start(
                        wht[:], wiq_hi[:, c0:c0 + 128].rearrange(
                            "(k p) c -> p k c", p=128))
                    wlt = st.tile([128, KT_Q, 128], f16, tag="wiql")
                    nc.sync.dma_start(
                        wlt[:], wiq_lo[:, c0:c0 + 128].rearrange(
                            "(k p) c -> p k c", p=128))
                    for kt in range(KT_Q):
                        nc.tensor.matmul(ps[:], wht[:, kt, :], qhi[:, kt, :],
                                         start=(kt == 0), stop=False)
                        nc.tensor.matmul(ps[:], wht[:, kt, :], qlo[:, kt, :],
                                         start=False, stop=False)
                        nc.tensor.matmul(ps[:], wlt[:, kt, :], qhi[:, kt, :],
                                         start=False, stop=(kt == KT_Q - 1))
                    qir = tp.tile([64, R], f32, tag="qir")
                    nc.scalar.copy(qir[:], ps[0:64, :])
                    ta = tp.tile([32, R], f32, tag="rta")
                    tb = tp.tile([32, R], f32, tag="rtb")
                    td = tp.tile([32, R], f32, tag="rtd")
                    te = tp.tile([32, R], f32, tag="rte")
                    x1, x2 = qir[0:32, :], qir[32:64, :]
                    nc.vector.tensor_mul(ta[:], x1, cosT_s[0:32, :])
                    nc.vector.tensor_mul(tb[:], x2, cosT_s[32:64, :])
                    nc.vector.tensor_mul(td[:], x2, sinT_s[32:64, :])
                    nc.vector.tensor_mul(te[:], x1, sinT_s[0:32, :])
                    nc.vector.tensor_sub(x1, ta[:], td[:])
                    nc.vector.tensor_add(ta[:], tb[:], te[:])
                    nc.vector.tensor_copy(x2, ta[:])
                    qih = tp.tile([128, R], f16, tag="qih")
                    qil = tp.tile([128, R], f16, tag="qil")
                    nc.vector.tensor_copy(qih[0:64, :], qir[:])
                    nc.vector.tensor_sub(qil[0:64, :], qir[:], qih[0:64, :])
                    nc.vector.tensor_copy(qih[64:128, :], ps[64:128, :])
                    nc.vector.tensor_sub(qil[64:128, :], ps[64:128, :], qih[64:128, :])
                    for rt in range(2):
                        lq_hi = qih[:, rt * 128:(rt + 1) * 128]
                        lq_lo = qil[:, rt * 128:(rt + 1) * 128]
                        for jc in range(2):
                            pl = ps_lg.tile([128, 1024], f32, tag="lg")
                            for hf in range(2):
                                js = jc * 1024 + hf * 512
                                sl = pl[:, hf * 512:(hf + 1) * 512]
                                nc.tensor.matmul(sl, lq_hi, khi[:, js:js + 512],
                                                 start=True, stop=False)
                                nc.tensor.matmul(sl, lq_hi, klo[:, js:js + 512],
                                                 start=False, stop=False)
                                nc.tensor.matmul(sl, lq_lo, khi[:, js:js + 512],
                                                 start=False, stop=True)
                            so = scores[rt][:, jc * 1024:(jc + 1) * 1024]
                            nc.vector._custom_dve(
                                ACC_W_RELU, out=so, in0=pl[:], in1=so,
                                s0=w_sb[:, rt, h:h + 1])

            # ---- D: threshold search + masks + maskT ----
            with tc.tile_pool(name="ps_tr", bufs=2, space="PSUM") as ps_tr:
                for rt in range(2):
                    sc = scores[rt]
                    rmax = tp.tile([128, 1], f32, tag="rmax")
                    rmin = tp.tile([128, 1], f32, tag="rmin")
                    nc.vector.tensor_reduce(rmax[:], sc[:], AX.X, OP.max)
                    nc.vector.tensor_reduce(rmin[:], sc[:], AX.X, OP.min)
                    nc.vector._custom_dve(MASKSEL, out=sc[:], in0=sc[:],
                                          in1=caus_s[rt][:], s0=NEG)
                    lo = tp.tile([128, 1], f32, tag="lo")
                    hi = tp.tile([128, 1], f32, tag="hi")
                    rng = tp.tile([128, 1], f32, tag="rng")
                    nc.vector.tensor_sub(rng[:], rmax[:], rmin[:])
                    nc.vector.tensor_scalar(lo[:], rng[:], -1e-3, -1e-6, OP.mult, OP.add)
                    nc.vector.tensor_add(lo[:], lo[:], rmin[:])
                    nc.vector.tensor_copy(hi[:], rmax[:])
                    cnt = tp.tile([128, 1], f32, tag="cnt")
                    junk = tp.tile([128, T], f16, tag="junk")
                    cge = tp.tile([128, 1], dt.uint32, tag="cge")
                    clt = tp.tile([128, 1], dt.uint32, tag="clt")
                    mid = tp.tile([128, 1], f32, tag="mid")
                    for _ in range(SEARCH_ITERS):
                        nc.vector.tensor_sub(mid[:], hi[:], lo[:])
                        nc.vector.tensor_scalar(mid[:], mid[:], 0.5, lo[:],
                                                OP.mult, OP.add)
                        nc.vector.tensor_scalar(junk[:], sc[:], mid[:], 0.0,
                                                OP.is_ge, OP.add,
                                                accum_out=cnt[:])
                        nc.vector.tensor_scalar(cge[:], cnt[:], TOPK - 0.5, None,
                                                OP.is_ge)
                        nc.vector.tensor_scalar(clt[:], cnt[:], TOPK - 0.5, None,
                                                OP.is_lt)
                        nc.vector.copy_predicated(lo[:], cge[:], mid[:])
                        nc.vector.copy_predicated(hi[:], clt[:], mid[:])
                    nc.vector.tensor_scalar(mask_r[rt][:], sc[:], lo[:], None, OP.is_ge)
                    nc.vector.tensor_mul(mask_r[rt][:], mask_r[rt][:], caus_s[rt][:])
                for jt in range(16):
                    for rt in range(2):
                        pt = ps_tr.tile([128, 128], f16, tag="mtr")
                        nc.tensor.transpose(pt[:], mask_r[rt][:, jt * 128:(jt + 1) * 128],
                                            id16[:])
                        nc.scalar.copy(maskT[:, jt, rt * 128:(rt + 1) * 128], pt[:])

        # ================= phase 2: attention =================
        with tc.tile_pool(name="p2", bufs=1) as p2, \
             tc.tile_pool(name="str2", bufs=4) as st2, \
             tc.tile_pool(name="tmp2", bufs=2) as tp2, \
             tc.tile_pool(name="dram2", bufs=1, space="DRAM") as drp2:
            qT = p2.tile([128, 24, R], f16)
            qR = p2.tile([64, H, R], f16)
            kvTk = p2.tile([128, 16, T], f16)
            o_sb = p2.tile([128, 16, R], f16)
            vdram = drp2.tile([T, H * VD], f16)

            with tc.tile_pool(name="ps_qp", bufs=3, space="PSUM") as ps_qp:
                for ft in range(24):
                    ps = ps_qp.tile([128, R], f32, tag="qp")
                    wt = st2.tile([128, KT_Q, 128], f16, tag="wqt")
                    nc.sync.dma_start(
                        wt[:], wq[:, ft * 128:(ft + 1) * 128].rearrange(
                            "(k p) c -> p k c", p=128))
                    for kt in range(KT_Q):
                        nc.tensor.matmul(ps[:], wt[:, kt, :], qhi[:, kt, :],
                                         start=(kt == 0), stop=(kt == KT_Q - 1))
                    nc.scalar.copy(qT[:, ft, :], ps[:])
                for hp in range(8):
                    for sub in range(2):
                        b1, b2 = sub * 64, sub * 64 + 32
                        x1 = qT[b1:b1 + 32, 16 + hp, :]
                        x2 = qT[b2:b2 + 32, 16 + hp, :]
                        ta = tp2.tile([32, R], f16, tag="qpa")
                        tb = tp2.tile([32, R], f16, tag="qpb")
                        td = tp2.tile([32, R], f16, tag="qpd")
                        te = tp2.tile([32, R], f16, tag="qpe")
                        nc.vector.tensor_mul(ta[:], x1, cosT_s[b1:b1 + 32, :])
                        nc.vector.tensor_mul(tb[:], x2, cosT_s[b2:b2 + 32, :])
                        nc.vector.tensor_mul(td[:], x2, sinT_s[b2:b2 + 32, :])
                        nc.vector.tensor_mul(te[:], x1, sinT_s[b1:b1 + 32, :])
                        nc.vector.tensor_sub(x1, ta[:], td[:])
                        nc.vector.tensor_add(ta[:], tb[:], te[:])
                        nc.vector.tensor_copy(x2, ta[:])
                        nc.vector.tensor_copy(
                            qR[:, 2 * hp + sub, :],
                            qT[b1:b1 + 64, 16 + hp, :])

            with tc.tile_pool(name="ps_kv", bufs=3, space="PSUM") as ps_kv:
                for ft in range(16):
                    for ncn in range(4):
                        ps = ps_kv.tile([128, 512], f32, tag="kv")
                        for kt in range(KT_KV):
                            wt = st2.tile([128, 128], f16, tag="wkvk")
                            nc.sync.dma_start(wt[:], wkv_k[kt * 128:(kt + 1) * 128,
                                                           ft * 128:(ft + 1) * 128])
                            nc.tensor.matmul(ps[:], wt[:],
                                             kvn[:, kt, ncn * 512:(ncn + 1) * 512],
                                             start=(kt == 0), stop=(kt == KT_KV - 1))
                        nc.scalar.copy(kvTk[:, ft, ncn * 512:(ncn + 1) * 512], ps[:])
                for mt in range(16):
                    for ncn in range(4):
                        ps = ps_kv.tile([128, 512], f32, tag="kv")
                        for kt in range(KT_KV):
                            wt = st2.tile([128, 512], f16, tag="wkvv")
                            nc.sync.dma_start(wt[:], wkv_v[kt * 128:(kt + 1) * 128,
                                                           ncn * 512:(ncn + 1) * 512])
                            nc.tensor.matmul(ps[:], kvn[:, kt, mt * 128:(mt + 1) * 128],
                                             wt[:], start=(kt == 0),
                                             stop=(kt == KT_KV - 1))
                        vt = tp2.tile([128, 512], f16, tag="vev")
                        nc.scalar.copy(vt[:], ps[:])
                        nc.sync.dma_start(vdram[mt * 128:(mt + 1) * 128,
                                                ncn * 512:(ncn + 1) * 512], vt[:])

            with tc.tile_pool(name="ps_att", bufs=1, space="PSUM") as ps_att, \
                 tc.tile_pool(name="ps_o", bufs=1, space="PSUM") as ps_o, \
                 tc.tile_pool(name="ps_s", bufs=1, space="PSUM") as ps_s:
                for g in range(4):
                    o_ps = ps_o.tile([128, 4, 512], f32, tag="o")
                    s_ps = ps_s.tile([128, 4 * R], f32, tag="s")
                    for jt in range(16):
                        att = ps_att.tile([128, 4 * R], f32, tag="att")
                        for i in range(4):
                            h = g * 4 + i
                            sl = att[:, i * R:(i + 1) * R]
                            nc.tensor.matmul(sl, kvTk[:, h, jt * 128:(jt + 1) * 128],
                                             qT[:, h, :], start=True, stop=False)
                            nc.tensor.matmul(
                                sl, kpe_s[:, jt * 128:(jt + 1) * 128],
                                qR[:, h, :], start=False, stop=True)
                        for i in range(4):
                            asl = att[:, i * R:(i + 1) * R]
                            nc.vector._custom_dve(
                                MASKSEL, out=asl, in0=asl,
                                in1=maskT[:, jt, :], s0=-60.0)
                        ee = tp2.tile([128, 4 * R], f16, tag="ee")
                        nc.scalar.activation(ee[:], att[:], AF.Exp)
                        vt = st2.tile([128, H * VD], f16, tag="vt")
                        nc.sync.dma_start(vt[:], vdram[jt * 128:(jt + 1) * 128, :])
                        for i in range(4):
                            h = g * 4 + i
                            nc.tensor.matmul(o_ps[:, i, 0:R],
                                             vt[:, h * 128:(h + 1) * 128],
                                             ee[:, i * R:(i + 1) * R],
                                             start=(jt == 0), stop=(jt == 15))
                        for sc2 in range(2):
                            nc.tensor.matmul(s_ps[:, sc2 * 512:(sc2 + 1) * 512],
                                             ones16[:],
                                             ee[:, sc2 * 512:(sc2 + 1) * 512],
                                             start=(jt == 0), stop=(jt == 15))
                    rec = tp2.tile([128, 4 * R], f32, tag="rec")
                    nc.vector.reciprocal(rec[:], s_ps[:])
                    for i in range(4):
                        h = g * 4 + i
                        nc.vector.tensor_mul(o_sb[:, h, :], o_ps[:, i, 0:R],
                                             rec[:, i * R:(i + 1) * R])

            with tc.tile_pool(name="ps_wo", bufs=2, space="PSUM") as ps_wo:
                for rt in range(2):
                    for ncn in range(14):
                        ps = ps_wo.tile([128, 512], f32, tag="wo")
                        for h in range(16):
                            wt = st2.tile([128, 512], f16, tag="wot")
                            nc.sync.dma_start(wt[:], wo[h * 128:(h + 1) * 128,
                                                        ncn * 512:(ncn + 1) * 512])
                            nc.tensor.matmul(ps[:], o_sb[:, h, rt * 128:(rt + 1) * 128],
                                             wt[:], start=(h == 0), stop=(h == 15))
                        ot = tp2.tile([128, 512], f32, tag="ot")
                        nc.scalar.copy(ot[:], ps[:])
                        nc.sync.dma_start(
                            out_d[rt * 128:(rt + 1) * 128,
                                  ncn * 512:(ncn + 1) * 512], ot[:])
        cst.release()
    nc.finalize()
    return nc


def _noop():
    from contextlib import nullcontext
    return nullcontext()


_NC_CACHE = None


def _host_prep(inputs):
    f32 = np.float32
    pos = np.asarray(inputs["positions"]).astype(f32)
    inv = 1.0 / (ROPE_BASE ** (np.arange(0, ROPE, 2, dtype=f32) / ROPE))
    ang = pos[:, None] * inv
    cos, sin = np.cos(ang).astype(f32), np.sin(ang).astype(f32)
    kpe = np.asarray(inputs["k_pe"]).astype(f32)
    xe, xo = kpe[:, 0::2], kpe[:, 1::2]
    kper = np.concatenate([xe * cos - xo * sin, xo * cos + xe * sin], axis=1)
    kperT = np.ascontiguousarray(kper.T).astype(np.float16)
    qw = np.asarray(inputs["q_a_ln_w"]).astype(f32)
    kvw = np.asarray(inputs["kv_a_ln_w"]).astype(f32)
    wq = np.asarray(inputs["Wq_b"]).astype(f32) * qw[:, None] * SCALE
    wq3 = wq.reshape(QLR, H, NOPE + ROPE)
    nope_part = wq3[:, :, :NOPE].reshape(QLR, H * NOPE)
    rope_part = wq3[:, :, NOPE:]
    ev, od = rope_part[:, :, 0::2], rope_part[:, :, 1::2]
    rope_perm = np.concatenate([ev, od], axis=2).reshape(QLR, H * ROPE)
    wq_perm = np.ascontiguousarray(
        np.concatenate([nope_part, rope_perm], axis=1)).astype(np.float16)
    wkv = np.asarray(inputs["Wkv_b"]).astype(f32) * kvw[:, None]
    wkv3 = wkv.reshape(KVLR, H, NOPE + VD)
    wkv_k = np.ascontiguousarray(
        wkv3[:, :, :NOPE].transpose(0, 1, 2).reshape(KVLR, H * NOPE)).astype(np.float16)
    wkv_v = np.ascontiguousarray(
        wkv3[:, :, NOPE:].reshape(KVLR, H * VD)).astype(np.float16)
    wiq = np.asarray(inputs["Wiq"]).astype(f32) * qw[:, None]
    wiq_hi = wiq.astype(np.float16)
    wiq_lo = (wiq - wiq_hi.astype(f32)).astype(np.float16)
    wikww = np.ascontiguousarray(np.concatenate(
        [np.asarray(inputs["Wik"]).astype(f32),
         np.asarray(inputs["Ww"]).astype(f32)], axis=1))
    wo = np.asarray(inputs["Wo"]).astype(np.float16)
    qcT = np.ascontiguousarray(np.asarray(inputs["q_c"]).astype(f32).T)
    kvcT = np.ascontiguousarray(np.asarray(inputs["kv_c"]).astype(f32).T)
    hTf = np.ascontiguousarray(np.asarray(inputs["hidden"]).astype(f32).T)
    tri01 = np.tril(np.ones((128, 128), np.float16))
    idm = np.eye(128, dtype=f32)
    gam_rep = np.ascontiguousarray(
        np.broadcast_to(np.asarray(inputs["ik_gamma"]).astype(f32), (128, ID)))
    bet_rep = np.ascontiguousarray(
        np.broadcast_to(np.asarray(inputs["ik_beta"]).astype(f32), (128, ID)))
    per_core = []
    for c in range(NC):
        r0 = c * R
        rows = slice(r0, r0 + R)
        causal = np.zeros((2, 128, T), np.float16)
        for rt in range(2):
            gb = r0 + rt * 128
            causal[rt, :, :gb] = 1.0
            causal[rt, :, gb:gb + 128] = tri01
        per_core.append(dict(
            qcT=np.ascontiguousarray(qcT[:, rows]),
            kvcT=kvcT, hT=np.ascontiguousarray(hTf[:, rows]),
            wikww=wikww, wiq_hi=wiq_hi, wiq_lo=wiq_lo, wq=wq_perm,
            wkv_k=wkv_k, wkv_v=wkv_v, wo=wo, kperT=kperT,
            cosT=np.ascontiguousarray(np.tile(cos[rows].T, (4, 1))),
            sinT=np.ascontiguousarray(np.tile(sin[rows].T, (4, 1))),
            cosK=np.ascontiguousarray(cos[rows]),
            sinK=np.ascontiguousarray(sin[rows]),
            gam_rep=gam_rep, bet_rep=bet_rep,
            idf32=idm, idf16=idm.astype(np.float16),
            causal01=causal,
        ))
    return per_core


def kernel(**inputs):
    global _NC_CACHE
    if _NC_CACHE is None:
        _NC_CACHE = build_nc()
    in_maps = _host_prep(inputs)
    res = run_bass_kernel_spmd(_NC_CACHE, in_maps, list(range(NC)))
    out = np.concatenate([res.results[c]["out"] for c in range(NC)], axis=0)
    return np.ascontiguousarray(out.astype(np.float32))



# revision 39
# speedup vs baseline: 2.7313x; 2.7313x over previous
"""DeepseekV3.2 sparse MLA attention — Trainium2 Bass kernel, 8-core SPMD.

Sharding: queries row-sharded (256 rows/core); keys/values replicated.
v3 design:
  - indexer q_idx / logits: single-pass f16 GEMMs (top-k tolerates ~1e-2),
    chunk-interleaved so DVE score accumulation starts early
  - k_idx computed transposed ([ID, tok]) -> no PE transposes; f16 AllGather
  - weights loaded once via large host-pre-laid-out DMAs (descriptors >= 512B)
  - top-k threshold bisection with both row-blocks in parallel: rt0 counting
    on DVE (is_ge+accum), rt1 counting on Act (Sign+accum)
  - attention: keys-on-partitions; k_nope^T/v generated per head-group chunk
    inside the attention loop (small SBUF footprint, no serial kv phase);
    mask applied as identity-matmul bias add; each PV accumulator in its own
    PSUM bank (a sibling group's start=True zeroes a whole bank region)
"""
import numpy as np

import concourse.bass as bass
import concourse.bacc as bacc
import concourse.mybir as mybir
import concourse.tile as tile
import concourse.dve_ops as dve_ops_mod
from concourse.bass_utils import run_bass_kernel_spmd
from concourse.dve_spec import Spec, Src0, Src1, C0, relu, select, lower, spec_leaves
from concourse.dve_ops import DveOp, OPS, get_dve_sub_opcode
from concourse.dve_uop import DveOpSpec

dt = mybir.dt
AF = mybir.ActivationFunctionType
OP = mybir.AluOpType
AX = mybir.AxisListType

T, HID, H, NOPE, ROPE, VD = 2048, 7168, 16, 128, 64, 128
QLR, KVLR, IH, ID, TOPK = 1536, 512, 32, 128, 512
EPS = 1e-6
SCALE = float((NOPE + ROPE) ** -0.5)
ROPE_BASE = 10000.0
NC = 8
R = T // NC            # 256 local query rows
NEG = -3.0e38
SEARCH_ITERS = 20
KT_H, KT_Q, KT_KV = HID // 128, QLR // 128, KVLR // 128


def _register(op):
    for o in OPS:
        if o.name == op.name:
            return o
    OPS.append(op)
    dve_ops_mod.CUSTOM_DVE_SPECS[op.name] = op.spec
    dve_ops_mod._SUB_OPCODE_FOR_NAME[op.name] = (
        dve_ops_mod._CUSTOM_DVE_ROW_BASE + len(OPS) - 1)
    for ver in ("v3", "v4"):
        sp = DveOpSpec(name=op.name, opcode=get_dve_sub_opcode(op.name),
                       uops=lower(op.spec, ver=ver),
                       rd1_en=Src1 in spec_leaves(op.spec))
        op.uops_sha[ver] = sp.sha(ver)
    return op


ACC_W_RELU = _register(DveOp(
    "ACC_W_RELU",
    Spec(body=Src1 + C0 * relu(Src0),
         reference=lambda in0, in1, s0, s1, imm2: in1 + s0 * np.maximum(in0, 0)),
    subdim=False, uops_sha={}))

MASKSEL = _register(DveOp(
    "MASKSEL",
    Spec(body=select(Src1, Src0, C0),
         reference=lambda in0, in1, s0, s1, imm2: np.where(in1 != 0, in0, s0)),
    subdim=False, uops_sha={}))


def build_nc():
    nc = bacc.Bacc(None)
    f32, f16 = dt.float32, dt.float16

    def din(name, shape, d=f32):
        return nc.dram_tensor(name, list(shape), d, kind="ExternalInput")

    qcT = din("qcT", [128, KT_Q, R])                 # q_c^T tiles, f32
    kvcT = din("kvcT", [128, KT_KV, T])              # kv_c^T tiles, f32
    hT = din("hT", [128, KT_H, R], f16)              # hidden^T tiles
    wikww = din("wikww", [128, KT_H, ID + IH], f16)  # [Wik|Ww] tiles
    wiq = din("wiq", [4, 128, KT_Q, IH * ID // 4], f16)   # 4 chunks x 8 heads
    wq = din("wq", [128, KT_Q, H * (NOPE + ROPE)], f16)
    wkv_k = din("wkv_k", [128, KT_KV, H * NOPE], f16)
    wkv_v = din("wkv_v", [128, KT_KV, H * VD], f16)
    wo = din("wo", [14, 128, H * 512], f16)          # 14 col-blocks of 512
    kperT = din("kperT", [ROPE, T], f16)
    cosT = din("cosT", [128, R])                     # f32, 4x32 stacked
    sinT = din("sinT", [128, R])
    cosQI = din("cosQI", [64, 4, R])                 # f32, replicated, 2x32 rows
    sinQI = din("sinQI", [64, 4, R])
    cosQR = din("cosQR", [128, 8, R], f16)           # replicated for q_pe rope
    sinQR = din("sinQR", [128, 8, R], f16)
    gamb = din("gamb", [128, 2])                     # [gamma | beta] f32
    idf16 = din("idf16", [128, 128], f16)
    caus_d = din("causal01", [2, 128, T], f16)
    out_d = nc.dram_tensor("out", [R, HID], f32, kind="ExternalOutput")

    with tile.TileContext(nc) as tc:
        # ---------- persistent pools (LIFO release order) ----------
        cst = tc.alloc_tile_pool(name="cst", bufs=1)
        ones32 = cst.tile([128, 1], f32); nc.vector.memset(ones32[:], 1.0)
        onesr = cst.tile([1, 128], f32); nc.vector.memset(onesr[:], 1.0)
        ones16 = cst.tile([128, 128], f16); nc.vector.memset(ones16[:], 1.0)
        id16 = cst.tile([128, 128], f16); nc.sync.dma_start(id16[:], idf16[:])
        eps1 = cst.tile([1, 1], f32); nc.vector.memset(eps1[:], EPS)
        qT = cst.tile([128, 24, R], f16)             # q_nope^T + q_pe^T tiles
        qR = cst.tile([64, H, R], f16)               # roped q_pe per head
        mb = cst.tile([128, 16, R], f16)             # mask bias {0,-60}
        o_sb = cst.tile([128, H, R], f16)

        pnorm = tc.alloc_tile_pool(name="pnorm", bufs=1)
        kvn = pnorm.tile([128, KT_KV, T], f16)       # rmsnorm(kv_c) f16

        psel = tc.alloc_tile_pool(name="psel", bufs=1)   # scores
        scores = [psel.tile([128, T], f32, tag=f"sc{i}", name=f"sc{i}")
                  for i in range(2)]
        pqn = tc.alloc_tile_pool(name="pqn", bufs=1)
        qn16 = pqn.tile([128, KT_Q, R], f16)         # rmsnorm(q_c) f16

        pKVN = tc.alloc_tile_pool(name="pKVN", bufs=1)
        kvcT_s = pKVN.tile([128, KT_KV, T], f32)
        rrep2 = pKVN.tile([128, T], f32)

        # phase-1 state released after logits (C)
        pkq = tc.alloc_tile_pool(name="pkq", bufs=1)
        w_sb = pkq.tile([128, 2, IH], f32)           # indexer w per token
        cosT_s = pkq.tile([128, R], f32); nc.sync.dma_start(cosT_s[:], cosT[:])
        sinT_s = pkq.tile([128, R], f32); nc.sync.dma_start(sinT_s[:], sinT[:])
        k16 = pkq.tile([128, T], f16)
        cqi = pkq.tile([64, 4, R], f32); nc.sync.dma_start(cqi[:], cosQI[:])
        sqi = pkq.tile([64, 4, R], f32); nc.sync.dma_start(sqi[:], sinQI[:])

        # ============ phase 1: indexer ============
        with tc.tile_pool(name="dr1", bufs=1, space="DRAM") as drp:
            inb = drp.tile([128, R], f16)
            outb = drp.tile([NC * 128, R], f16)

            # ---- A: k_idx (transposed) + w_idx + AllGather ----
            with tc.tile_pool(name="pA", bufs=1) as pA, \
                 tc.tile_pool(name="tpA", bufs=2) as tpA, \
                 tc.tile_pool(name="ps_kw", bufs=1, space="PSUM") as ps_kw:
                hT_s = pA.tile([128, KT_H, R], f16)
                wik_s = pA.tile([128, KT_H, ID + IH], f16)
                for pc in range(4):
                    ks = slice(pc * 14, (pc + 1) * 14)
                    nc.sync.dma_start(hT_s[:, ks, :], hT[:, ks, :])
                    nc.sync.dma_start(wik_s[:, ks, :], wikww[:, ks, :])
                gamb_s = pA.tile([128, 2], f32)
                nc.sync.dma_start(gamb_s[:], gamb[:])

                kps = ps_kw.tile([128, R], f32, tag="kps")
                for kt in range(KT_H):
                    nc.tensor.matmul(kps[:], wik_s[:, kt, 0:ID], hT_s[:, kt, :],
                                     start=(kt == 0), stop=(kt == KT_H - 1))
                for mt in range(2):
                    wps = ps_kw.tile([128, IH], f32, tag=f"wps{mt}")
                    for kt in range(KT_H):
                        nc.tensor.matmul(wps[:], hT_s[:, kt, mt * 128:(mt + 1) * 128],
                                         wik_s[:, kt, ID:],
                                         start=(kt == 0), stop=(kt == KT_H - 1))
                    nc.scalar.copy(w_sb[:, mt, :], wps[:])
                kid = tpA.tile([128, R], f32, tag="kid")
                nc.scalar.copy(kid[:], kps[:])
                # LN over partition dim via ones-matmuls
                sq = tpA.tile([128, R], f32, tag="ksq")
                ssum0 = ps_kw.tile([1, R], f32, tag="ssum0")
                ssum1 = ps_kw.tile([1, R], f32, tag="ssum1")
                nc.scalar.activation(sq[:], kid[:], AF.Square)
                nc.tensor.matmul(ssum0[:], ones32[:], kid[:], start=True, stop=True)
                nc.tensor.matmul(ssum1[:], ones32[:], sq[:], start=True, stop=True)
                mu = tpA.tile([1, R], f32, tag="mu")
                nc.vector.tensor_scalar_mul(mu[:], ssum0[:], 1.0 / ID)
                musq = tpA.tile([1, R], f32, tag="musq")
                nc.scalar.activation(musq[:], mu[:], AF.Square)
                var = tpA.tile([1, R], f32, tag="var")
                nc.vector.tensor_scalar_mul(var[:], ssum1[:], 1.0 / ID)
                nc.vector.tensor_sub(var[:], var[:], musq[:])
                std = tpA.tile([1, R], f32, tag="std")
                nc.scalar.activation(std[:], var[:], AF.Sqrt, bias=eps1[:])
                rstd = tpA.tile([1, R], f32, tag="rstd")
                nc.vector.reciprocal(rstd[:], std[:])
                bcp = ps_kw.tile([128, 2, R], f32, tag="bcp")
                nc.tensor.matmul(bcp[:, 0, :], onesr[:], mu[:], start=True, stop=True)
                nc.tensor.matmul(bcp[:, 1, :], onesr[:], rstd[:], start=True, stop=True)
                nc.vector.tensor_sub(kid[:], kid[:], bcp[:, 0, :])
                nc.vector.tensor_mul(kid[:], kid[:], bcp[:, 1, :])
                nc.vector.tensor_scalar(kid[:], kid[:], gamb_s[:, 0:1], None, OP.mult)
                nc.vector.tensor_scalar(kid[:], kid[:], gamb_s[:, 1:2], None, OP.add)
                # neox rope rows 0..63 (pairs r, r+32), f32; temps at base 0
                t1 = tpA.tile([32, R], f32, tag="kr1")
                t2 = tpA.tile([32, R], f32, tag="kr2")
                t3 = tpA.tile([32, R], f32, tag="kr3")
                t4 = tpA.tile([32, R], f32, tag="kr4")
                x1, x2 = kid[0:32, :], kid[32:64, :]
                nc.vector.tensor_mul(t1[:], x1, cosT_s[0:32, :])
                nc.vector.tensor_mul(t2[:], x2, sinT_s[32:64, :])
                nc.vector.tensor_mul(t3[:], x1, sinT_s[0:32, :])
                nc.vector.tensor_mul(t4[:], x2, cosT_s[32:64, :])
                nc.vector.tensor_sub(x1, t1[:], t2[:])
                nc.vector.tensor_add(x2, t3[:], t4[:])
                ki16 = tpA.tile([128, R], f16, tag="ki16")
                nc.vector.tensor_copy(ki16[:], kid[:])
                nc.scalar.dma_start(inb[:], ki16[:])
                nc.gpsimd.collective_compute(
                    "AllGather", OP.bypass, replica_groups=[list(range(NC))],
                    ins=[inb[:]], outs=[outb[:]])

            # ---- B: rmsnorm(q_c) and rmsnorm(kv_c) ----
            with tc.tile_pool(name="pB", bufs=1) as pB, \
                 tc.tile_pool(name="tpB", bufs=1) as tpB:
              with tc.tile_pool(name="ps_sm", bufs=1, space="PSUM") as ps_sm:
                qcT_s = pB.tile([128, KT_Q, R], f32)
                nc.sync.dma_start(qcT_s[:], qcT[:])
                sqp = ps_sm.tile([1, R], f32, tag="sqp")
                for kt in range(KT_Q):
                    sq2 = tpB.tile([128, R], f32, tag="qsq")
                    nc.scalar.activation(sq2[:], qcT_s[:, kt, :], AF.Square)
                    nc.tensor.matmul(sqp[:], ones32[:], sq2[:],
                                     start=(kt == 0), stop=(kt == KT_Q - 1))
                stdq = tpB.tile([1, R], f32, tag="stdq")
                nc.scalar.activation(stdq[:], sqp[:], AF.Sqrt,
                                     scale=1.0 / QLR, bias=eps1[:])
                rstdq = tpB.tile([1, R], f32, tag="rstdq")
                nc.vector.reciprocal(rstdq[:], stdq[:])
                rp = ps_sm.tile([128, R], f32, tag="rp")
                nc.tensor.matmul(rp[:], onesr[:], rstdq[:], start=True, stop=True)
                rrep = tpB.tile([128, R], f32, tag="rrep")
                nc.scalar.copy(rrep[:], rp[:])
                for kt in range(KT_Q):
                    nc.vector.tensor_mul(qn16[:, kt, :], qcT_s[:, kt, :], rrep[:])

            # ---- B2 stats: rmsnorm(kv_c) scale factors (muls happen in C) ----
            with tc.tile_pool(name="tpB2", bufs=1) as tpB2, \
                 tc.tile_pool(name="ps_sv", bufs=1, space="PSUM") as ps_sv:
                nc.sync.dma_start(kvcT_s[:], kvcT[:])
                svp = ps_sv.tile([1, T], f32, tag="svp")
                for kt in range(KT_KV):
                    sq3 = tpB2.tile([128, T], f32, tag="kvsq")
                    nc.scalar.activation(sq3[:], kvcT_s[:, kt, :], AF.Square)
                    for jc in range(4):
                        nc.tensor.matmul(svp[:, jc * 512:(jc + 1) * 512], ones32[:],
                                         sq3[:, jc * 512:(jc + 1) * 512],
                                         start=(kt == 0), stop=(kt == KT_KV - 1))
                stdv = tpB2.tile([1, T], f32, tag="stdv")
                nc.scalar.activation(stdv[:], svp[:], AF.Sqrt,
                                     scale=1.0 / KVLR, bias=eps1[:])
                rstdv = tpB2.tile([1, T], f32, tag="rstdv")
                nc.vector.reciprocal(rstdv[:], stdv[:])
                rp2 = ps_sv.tile([128, T], f32, tag="rp2")
                for jc in range(4):
                    nc.tensor.matmul(rp2[:, jc * 512:(jc + 1) * 512], onesr[:],
                                     rstdv[:, jc * 512:(jc + 1) * 512],
                                     start=True, stop=True)
                nc.scalar.copy(rrep2[:], rp2[:])

            # ---- C: q_idx f16 + rope + logits + score accumulation,
            #         chunk-interleaved so DVE ACC starts early ----
            for i in range(2):
                nc.vector.memset(scores[i][:], 0.0)
            with tc.tile_pool(name="pC", bufs=1) as pC, \
                 tc.tile_pool(name="tpC", bufs=1) as tpC, \
                 tc.tile_pool(name="ps_qi", bufs=3, space="PSUM") as ps_qi, \
                 tc.tile_pool(name="ps_lg", bufs=2, space="PSUM") as ps_lg:
                wiq_pre = []
                for ch in range(1):
                    wt = pC.tile([128, KT_Q, 8 * ID], f16, tag="wiqc",
                                 name=f"wiqp{ch}")
                    nc.sync.dma_start(wt[:], wiq[ch])
                    wiq_pre.append(wt)
                # gathered k_idx lands after the first two wiq prefetches so
                # the in-order SP queue is not blocked behind the collective
                nc.sync.dma_start(
                    k16[:].rearrange("p (r c) -> p r c", r=NC),
                    outb[:].rearrange("(r p) c -> p r c", p=128))
                for ch in range(4):
                    if ch < 1:
                        wiq_sb = wiq_pre[ch]
                    else:
                        wiq_sb = pC.tile([128, KT_Q, 8 * ID], f16, tag="wiqc")
                        nc.sync.dma_start(wiq_sb[:], wiq[ch])
                    qi32 = pC.tile([128, 8, R], f32, tag=f"qi32_{ch % 2}",
                                   name=f"qi32_{ch}")
                    qi16 = pC.tile([128, 8, R], f16, tag=f"qi16_{ch % 2}",
                                   name=f"qi16_{ch}")
                    for h8 in range(8):
                        ps = ps_qi.tile([128, R], f32, tag="qi")
                        for kt in range(KT_Q):
                            nc.tensor.matmul(
                                ps[:], wiq_sb[:, kt, h8 * ID:(h8 + 1) * ID],
                                qn16[:, kt, :],
                                start=(kt == 0), stop=(kt == KT_Q - 1))
                        nc.scalar.copy(qi32[:, h8, :], ps[:])
                    # f32 neox rope, batched by 4 heads; the rope'd rows
                    # combine straight into qi16 (f16 out from f32 ins)
                    eng = nc.gpsimd
                    for hb in range(2):
                        hs = slice(hb * 4, hb * 4 + 4)
                        x1 = qi32[0:32, hs, :]
                        x2 = qi32[32:64, hs, :]
                        tt1 = tpC.tile([32, 4, R], f32, tag="qro1")
                        tt2 = tpC.tile([32, 4, R], f32, tag="qro2")
                        tt3 = tpC.tile([32, 4, R], f32, tag="qro3")
                        tt4 = tpC.tile([32, 4, R], f32, tag="qro4")
                        eng.tensor_mul(tt1[:], x1, cqi[0:32, :, :])
                        eng.tensor_mul(tt2[:], x2, sqi[32:64, :, :])
                        eng.tensor_mul(tt3[:], x1, sqi[0:32, :, :])
                        eng.tensor_mul(tt4[:], x2, cqi[32:64, :, :])
                        eng.tensor_sub(qi16[0:32, hs, :], tt1[:], tt2[:])
                        eng.tensor_add(qi16[32:64, hs, :], tt3[:], tt4[:])
                    nc.scalar.copy(qi16[64:128, :, :], qi32[64:128, :, :])
                    if ch == 1:
                        for kt in range(KT_KV):
                            nc.gpsimd.tensor_mul(kvn[:, kt, :],
                                                 kvcT_s[:, kt, :], rrep2[:])
                    for h8 in range(8):
                        h = ch * 8 + h8
                        for rt in range(2):
                            lq = qi16[:, h8, rt * 128:(rt + 1) * 128]
                            for half in range(2):
                                pl = ps_lg.tile([128, 1024], f32, tag="lg")
                                for jc in range(2):
                                    ko = half * 1024 + jc * 512
                                    nc.tensor.matmul(
                                        pl[:, jc * 512:(jc + 1) * 512], lq,
                                        k16[:, ko:ko + 512],
                                        start=True, stop=True)
                                nc.vector._custom_dve(
                                    ACC_W_RELU,
                                    out=scores[rt][:, half * 1024:(half + 1) * 1024],
                                    in0=pl[:],
                                    in1=scores[rt][:, half * 1024:(half + 1) * 1024],
                                    s0=w_sb[:, rt, h:h + 1])

        pkq.release()
        pKVN.release()

        # ---- q projection + q_pe rope (overlaps score accumulation tail) ----
        with tc.tile_pool(name="wqp", bufs=1) as wqp, \
             tc.tile_pool(name="ps_qp", bufs=3, space="PSUM") as ps_qp:
            cqr = wqp.tile([128, 8, R], f16); nc.sync.dma_start(cqr[:], cosQR[:])
            sqr = wqp.tile([128, 8, R], f16); nc.sync.dma_start(sqr[:], sinQR[:])
            wq_sb = wqp.tile([128, KT_Q, H * (NOPE + ROPE)], f16)
            half = H * (NOPE + ROPE) // 2
            nc.sync.dma_start(wq_sb[:, :, 0:half], wq[:, :, 0:half])
            nc.sync.dma_start(wq_sb[:, :, half:], wq[:, :, half:])
            for ft in range(24):
                ps = ps_qp.tile([128, R], f32, tag="qp")
                for kt in range(KT_Q):
                    nc.tensor.matmul(ps[:], wq_sb[:, kt, ft * 128:(ft + 1) * 128],
                                     qn16[:, kt, :],
                                     start=(kt == 0), stop=(kt == KT_Q - 1))
                nc.scalar.copy(qT[:, ft, :], ps[:])
            rope = qT[:, 16:24, :]
            for b in (0, 64):
                x1 = rope[b:b + 32, :, :]
                x2 = rope[b + 32:b + 64, :, :]
                ta = wqp.tile([32, 8, R], f16, tag="qpa")
                tb = wqp.tile([32, 8, R], f16, tag="qpb")
                td = wqp.tile([32, 8, R], f16, tag="qpd")
                te = wqp.tile([32, 8, R], f16, tag="qpe")
                nc.vector.tensor_mul(ta[:], x1, cqr[b:b + 32, :, :])
                nc.vector.tensor_mul(tb[:], x2, sqr[b + 32:b + 64, :, :])
                nc.vector.tensor_mul(td[:], x1, sqr[b:b + 32, :, :])
                nc.vector.tensor_mul(te[:], x2, cqr[b + 32:b + 64, :, :])
                nc.vector.tensor_sub(x1, ta[:], tb[:])
                nc.vector.tensor_add(x2, td[:], te[:])
            for hp in range(8):
                nc.vector.tensor_copy(qR[:, 2 * hp, :], rope[0:64, hp, :])
                nc.vector.tensor_copy(qR[:, 2 * hp + 1, :], rope[64:128, hp, :])

        pqn.release()

        # ============ phase 2 pools open early: kv chunk for g0 is generated
        # during the bisection (PE would otherwise idle) ============
        with tc.tile_pool(name="wkvp", bufs=1) as wkvp, \
             tc.tile_pool(name="pchunk", bufs=2) as pchunk, \
             tc.tile_pool(name="patt", bufs=2) as patt, \
             tc.tile_pool(name="ps_att", bufs=2, space="PSUM") as ps_att:
            wkvk_sb = wkvp.tile([128, KT_KV, H * NOPE], f16)
            nc.sync.dma_start(wkvk_sb[:], wkv_k[:])
            wkvv_sb = wkvp.tile([128, KT_KV, H * VD], f16)
            nc.sync.dma_start(wkvv_sb[:], wkv_v[:])
            kpe_s = wkvp.tile([ROPE, T], f16)
            nc.sync.dma_start(kpe_s[:], kperT[:])

            def kv_chunk(g):
                # this head-group's k_nope^T [128, 4, T] and v [128, 16, 512]
                kTg = pchunk.tile([128, 4, T], f16, tag="kTg", name=f"kTg{g}")
                vg = pchunk.tile([128, 16, 512], f16, tag="vg", name=f"vg{g}")
                for f4 in range(4):
                    ft = g * 4 + f4
                    for ncn in range(4):
                        ps = ps_att.tile([128, 512], f32, tag="att")
                        for kt in range(KT_KV):
                            nc.tensor.matmul(
                                ps[:], wkvk_sb[:, kt, ft * 128:(ft + 1) * 128],
                                kvn[:, kt, ncn * 512:(ncn + 1) * 512],
                                start=(kt == 0), stop=(kt == KT_KV - 1))
                        nc.scalar.copy(kTg[:, f4, ncn * 512:(ncn + 1) * 512], ps[:])
                for mt in range(16):
                    ps = ps_att.tile([128, 512], f32, tag="att")
                    for kt in range(KT_KV):
                        nc.tensor.matmul(
                            ps[:], kvn[:, kt, mt * 128:(mt + 1) * 128],
                            wkvv_sb[:, kt, g * 512:(g + 1) * 512],
                            start=(kt == 0), stop=(kt == KT_KV - 1))
                    nc.scalar.copy(vg[:, mt, :], ps[:])
                return kTg, vg

            chunks = {0: kv_chunk(0), 1: kv_chunk(1)}

            # ---- D: threshold bisection; rt0 counts on DVE, rt1 on Act ----
            with tc.tile_pool(name="pD", bufs=1) as pD, \
                 tc.tile_pool(name="ps_tr", bufs=2, space="PSUM") as ps_tr:
                caus_s = [pD.tile([128, T], f16, tag=f"cz{i}", name=f"cz{i}")
                          for i in range(2)]
                for rt in range(2):
                    nc.sync.dma_start(caus_s[rt][:], caus_d[rt])
                def pdt(nm, shape, d=f32):
                    return [pD.tile(shape, d, tag=f"{nm}{rt}", name=f"{nm}{rt}")
                            for rt in range(2)]
                lo = pdt("lo", [128, 1])
                rng = pdt("rng", [128, 1])
                mid = pdt("mid", [128, 1])
                nmid = pdt("nm", [128, 1])
                sacc = pdt("sa", [128, 1])
                cge = pdt("cg", [128, 1], dt.uint32)
                junk = pdt("jk", [128, T], f16)
                mbr = pdt("mb", [128, T], f16)
                for rt in range(2):
                    sc = scores[rt]
                    rmax = pD.tile([128, 1], f32, tag=f"rx{rt}")
                    rmin = pD.tile([128, 1], f32, tag=f"rn{rt}")
                    nc.vector.tensor_reduce(rmax[:], sc[:], AX.X, OP.max)
                    nc.vector.tensor_reduce(rmin[:], sc[:], AX.X, OP.min)
                    nc.vector._custom_dve(MASKSEL, out=sc[:], in0=sc[:],
                                          in1=caus_s[rt][:], s0=NEG)
                    nc.vector.tensor_sub(rng[rt][:], rmax[:], rmin[:])
                    nc.vector.tensor_scalar(lo[rt][:], rng[rt][:], -1e-3, -1e-6,
                                            OP.mult, OP.add)
                    nc.vector.tensor_add(lo[rt][:], lo[rt][:], rmin[:])
                    nc.vector.tensor_sub(rng[rt][:], rmax[:], lo[rt][:])
                for _ in range(SEARCH_ITERS):
                    # rt1 chain: count via Act Sign+accum
                    rt = 1
                    nc.vector.tensor_scalar_mul(rng[rt][:], rng[rt][:], 0.5)
                    nc.vector.tensor_add(mid[rt][:], lo[rt][:], rng[rt][:])
                    nc.vector.tensor_scalar_mul(nmid[rt][:], mid[rt][:], -1.0)
                    nc.scalar.activation(junk[rt][:], scores[rt][:], AF.Sign,
                                         bias=nmid[rt][:], accum_out=sacc[rt][:])
                    nc.vector.tensor_scalar(cge[rt][:], sacc[rt][:],
                                            float(2 * TOPK - T - 1), None, OP.is_ge)
                    nc.vector.copy_predicated(lo[rt][:], cge[rt][:], mid[rt][:])
                    # rt0 chain: count via DVE is_ge+accum
                    rt = 0
                    nc.vector.tensor_scalar_mul(rng[rt][:], rng[rt][:], 0.5)
                    nc.vector.tensor_add(mid[rt][:], lo[rt][:], rng[rt][:])
                    nc.vector.tensor_scalar(junk[rt][:], scores[rt][:], mid[rt][:],
                                            0.0, OP.is_ge, OP.add,
                                            accum_out=sacc[rt][:])
                    nc.vector.tensor_scalar(cge[rt][:], sacc[rt][:],
                                            TOPK - 0.5, None, OP.is_ge)
                    nc.vector.copy_predicated(lo[rt][:], cge[rt][:], mid[rt][:])
                for rt in range(2):
                    nc.vector.tensor_scalar(mbr[rt][:], scores[rt][:], lo[rt][:],
                                            -60.0, OP.is_lt, OP.mult)
                for jt in range(16):
                    for rt in range(2):
                        pt = ps_tr.tile([128, 128], f16, tag="mtr")
                        nc.tensor.transpose(pt[:],
                                            mbr[rt][:, jt * 128:(jt + 1) * 128],
                                            id16[:])
                        nc.scalar.copy(mb[:, jt, rt * 128:(rt + 1) * 128], pt[:])

            # ---- attention ----
            with tc.tile_pool(name="ps_o", bufs=1, space="PSUM") as ps_o, \
                 tc.tile_pool(name="ps_s", bufs=1, space="PSUM") as ps_s:
                for g in range(4):
                    kTg, vg = chunks[g] if g < 2 else kv_chunk(g)
                    # each PV accumulation group gets its own PSUM bank: a
                    # sibling group's start=True zeroes the whole bank region
                    o_ps = ps_o.tile([128, 4, 512], f32, tag="o")
                    s_ps = ps_s.tile([128, 4 * R], f32, tag="s")
                    for jt in range(16):
                        for pr in range(2):   # head pairs (2 per att bank tile)
                            att = ps_att.tile([128, 2 * R], f32, tag="att")
                            for i2 in range(2):
                                i = pr * 2 + i2
                                h = g * 4 + i
                                sl = att[:, i2 * R:(i2 + 1) * R]
                                nc.tensor.matmul(sl,
                                                 kTg[:, i, jt * 128:(jt + 1) * 128],
                                                 qT[:, h, :], start=True, stop=False)
                                nc.tensor.matmul(sl,
                                                 kpe_s[:, jt * 128:(jt + 1) * 128],
                                                 qR[:, h, :], start=False, stop=False)
                                nc.tensor.matmul(sl, id16[:], mb[:, jt, :],
                                                 start=False, stop=True)
                            ee = patt.tile([128, 2 * R], f16, tag="ee")
                            nc.scalar.activation(ee[:], att[:], AF.Exp)
                            for i2 in range(2):
                                i = pr * 2 + i2
                                nc.tensor.matmul(o_ps[:, i, 0:R],
                                                 vg[:, jt, i * 128:(i + 1) * 128],
                                                 ee[:, i2 * R:(i2 + 1) * R],
                                                 start=(jt == 0), stop=(jt == 15))
                            nc.tensor.matmul(s_ps[:, pr * 512:(pr + 1) * 512],
                                             ones16[:], ee[:],
                                             start=(jt == 0), stop=(jt == 15))
                    rec = patt.tile([128, 4 * R], f32, tag="rec")
                    nc.vector.reciprocal(rec[:], s_ps[:])
                    for i in range(4):
                        h = g * 4 + i
                        nc.vector.tensor_mul(o_sb[:, h, :], o_ps[:, i, 0:R],
                                             rec[:, i * R:(i + 1) * R])
        psel.release()
        pnorm.release()

        # ---- output projection ----
        with tc.tile_pool(name="wop", bufs=2) as wop, \
             tc.tile_pool(name="ps_wo", bufs=2, space="PSUM") as ps_wo:
            for cb in range(14):
                wos = wop.tile([128, H, 512], f16, tag="wos")
                nc.sync.dma_start(wos[:], wo[cb])
                for rt in range(2):
                    ps = ps_wo.tile([128, 512], f32, tag="wo")
                    for h in range(H):
                        nc.tensor.matmul(ps[:],
                                         o_sb[:, h, rt * 128:(rt + 1) * 128],
                                         wos[:, h, :], start=(h == 0),
                                         stop=(h == H - 1))
                    ot = wop.tile([128, 512], f32, tag="ot")
                    nc.scalar.copy(ot[:], ps[:])
                    nc.sync.dma_start(
                        out_d[rt * 128:(rt + 1) * 128,
                              cb * 512:(cb + 1) * 512], ot[:])
        cst.release()
    nc.finalize()
    return nc


_NC_CACHE = None


def _host_prep(inputs):
    f32, f16 = np.float32, np.float16
    pos = np.asarray(inputs["positions"]).astype(f32)
    inv = 1.0 / (ROPE_BASE ** (np.arange(0, ROPE, 2, dtype=f32) / ROPE))
    ang = pos[:, None] * inv
    cos, sin = np.cos(ang).astype(f32), np.sin(ang).astype(f32)
    kpe = np.asarray(inputs["k_pe"]).astype(f32)
    xe, xo = kpe[:, 0::2], kpe[:, 1::2]
    kper = np.concatenate([xe * cos - xo * sin, xo * cos + xe * sin], axis=1)
    kperT = np.ascontiguousarray(kper.T).astype(f16)
    qw = np.asarray(inputs["q_a_ln_w"]).astype(f32)
    kvw = np.asarray(inputs["kv_a_ln_w"]).astype(f32)

    def tiles(a, kt):
        # [kt*128, C] -> [128, kt, C]
        return np.ascontiguousarray(
            a.reshape(kt, 128, a.shape[1]).transpose(1, 0, 2))

    wqf = np.asarray(inputs["Wq_b"]).astype(f32) * qw[:, None] * SCALE
    wq3 = wqf.reshape(QLR, H, NOPE + ROPE)
    nope_part = wq3[:, :, :NOPE].reshape(QLR, H * NOPE)
    rope_part = wq3[:, :, NOPE:]
    ev, od = rope_part[:, :, 0::2], rope_part[:, :, 1::2]
    rope_perm = np.concatenate([ev, od], axis=2).reshape(QLR, H * ROPE)
    wq_perm = np.concatenate([nope_part, rope_perm], axis=1).astype(f16)
    wq_p = tiles(wq_perm, KT_Q)

    wkv = np.asarray(inputs["Wkv_b"]).astype(f32) * kvw[:, None]
    wkv3 = wkv.reshape(KVLR, H, NOPE + VD)
    wkv_k_p = tiles(wkv3[:, :, :NOPE].reshape(KVLR, H * NOPE).astype(f16), KT_KV)
    wkv_v_p = tiles(wkv3[:, :, NOPE:].reshape(KVLR, H * VD).astype(f16), KT_KV)

    wiqf = (np.asarray(inputs["Wiq"]).astype(f32) * qw[:, None]).astype(f16)
    wiq_t = tiles(wiqf, KT_Q)                       # [128, 12, 4096]
    wiq_p = np.ascontiguousarray(
        wiq_t.reshape(128, KT_Q, 4, 8 * ID).transpose(2, 0, 1, 3))

    wikww_f = np.concatenate(
        [np.asarray(inputs["Wik"]).astype(f32),
         np.asarray(inputs["Ww"]).astype(f32)], axis=1).astype(f16)
    wikww_p = tiles(wikww_f, KT_H)

    wof = np.asarray(inputs["Wo"]).astype(f16)      # [2048, 7168]
    wo_p = np.ascontiguousarray(
        wof.reshape(H, 128, 14, 512).transpose(2, 1, 0, 3).reshape(
            14, 128, H * 512))

    hTf = np.asarray(inputs["hidden"]).astype(f32).T.astype(f16)   # [7168, T]
    kvcTf = np.ascontiguousarray(np.asarray(inputs["kv_c"]).astype(f32).T)
    qcTf = np.ascontiguousarray(np.asarray(inputs["q_c"]).astype(f32).T)
    kvcT_p = tiles(kvcTf, KT_KV)

    tri01 = np.tril(np.ones((128, 128), f16))
    idm16 = np.eye(128, dtype=f16)
    gamb = np.ascontiguousarray(np.stack(
        [np.asarray(inputs["ik_gamma"]).astype(f32),
         np.asarray(inputs["ik_beta"]).astype(f32)], axis=1))

    per_core = []
    for c in range(NC):
        r0 = c * R
        rows = slice(r0, r0 + R)
        causal = np.zeros((2, 128, T), f16)
        for rt in range(2):
            gb = r0 + rt * 128
            causal[rt, :, :gb] = 1.0
            causal[rt, :, gb:gb + 128] = tri01
        cosl, sinl = cos[rows], sin[rows]           # [R, 32]
        cosT_c = np.ascontiguousarray(np.tile(cosl.T, (4, 1)))     # [128, R]
        sinT_c = np.ascontiguousarray(np.tile(sinl.T, (4, 1)))
        cqi_h = np.ascontiguousarray(np.broadcast_to(
            np.tile(cosl.T, (2, 1))[:, None, :], (64, 4, R)))
        sqi_h = np.ascontiguousarray(np.broadcast_to(
            np.tile(sinl.T, (2, 1))[:, None, :], (64, 4, R)))
        cqr = np.ascontiguousarray(
            np.broadcast_to(cosT_c[:, None, :], (128, 8, R))).astype(f16)
        sqr = np.ascontiguousarray(
            np.broadcast_to(sinT_c[:, None, :], (128, 8, R))).astype(f16)
        per_core.append(dict(
            qcT=tiles(np.ascontiguousarray(qcTf[:, rows]), KT_Q),
            kvcT=kvcT_p,
            hT=tiles(np.ascontiguousarray(hTf[:, rows]), KT_H),
            wikww=wikww_p, wiq=wiq_p, wq=wq_p,
            wkv_k=wkv_k_p, wkv_v=wkv_v_p, wo=wo_p, kperT=kperT,
            cosT=cosT_c, sinT=sinT_c,
            cosQI=cqi_h, sinQI=sqi_h, cosQR=cqr, sinQR=sqr,
            gamb=gamb, idf16=idm16, causal01=causal,
        ))
    return per_core


def kernel(**inputs):
    global _NC_CACHE
    if _NC_CACHE is None:
        _NC_CACHE = build_nc()
    in_maps = _host_prep(inputs)
    res = run_bass_kernel_spmd(_NC_CACHE, in_maps, list(range(NC)))
    out = np.concatenate([res.results[c]["out"] for c in range(NC)], axis=0)
    return np.ascontiguousarray(out.astype(np.float32))
